# revision 49
# baseline (speedup 1.0000x reference)
"""Multi-head attention (B=2, S=2048, D=2048, H=16) on 8 trn2 NeuronCores.

Sharding: tensor-parallel over heads. Core c owns heads [2c, 2c+1]:
  - q/k/v projections via error-compensated fp8 DoubleRow matmuls
    (x and W split into e4m3 hi+lo pairs on host; 3 of 4 cross terms
    on-chip => bf16-level accuracy at 0.75x the PE cycles)
  - attention for its 2 heads x 2 batches in bf16 (transposed-score
    layout, fused softmax denominator, no on-chip transposes)
  - partial output projection (fp8 DoubleRow 3-term, MT split on-chip)
    merged_c @ Wo[:, c_slice].T  -> [B, D, S] fp32
Host: sums the 8 partials, transposes, adds bo.

Scaling: x is shipped as fp8(16*x) hi/lo, weights as fp8(64*W) hi/lo;
the 1/1024 is folded into the PSUM-readout activation scale.
"""

import numpy as np

try:
    import concourse.bass as bass  # noqa: F401
except ImportError:  # pragma: no cover - fresh grading dir
    import sys

    sys.path.insert(0, "/opt/trn_rl_repo")

import ml_dtypes

import concourse.bacc as bacc
import concourse.mybir as mybir
import concourse.tile as tile
from concourse.bass_utils import run_bass_kernel_spmd

B, S, D, H = 2, 2048, 2048, 16
HD = D // H  # 128
N_CORES = 8
HPC = H // N_CORES  # heads per core = 2
CD = HPC * HD  # per-core projection dims = 256
TOK = B * S  # 4096

BF16 = mybir.dt.bfloat16
F8 = mybir.dt.float8e4
F32 = mybir.dt.float32
DR = mybir.MatmulPerfMode.DoubleRow

TT = 512  # token tile (free dim of most matmuls)
KC = D // 128  # contraction chunks for projections = 16
KC2 = KC // 2  # DoubleRow (256-deep) chunks = 8
NB = S // 128  # key blocks per batch = 16
NQ = S // TT  # q tiles per batch = 4
NT = S // TT  # token tiles per batch = 4
SCALE = 1.0 / float(np.sqrt(HD))
SX = 16.0  # host pre-scale on x
SW = 64.0  # host pre-scale on weights
INV_XW = 1.0 / (SX * SW)
SM = 64.0  # on-chip pre-scale on merged (MT)
INV_MW = 1.0 / (SM * SW)

Act = mybir.ActivationFunctionType
Alu = mybir.AluOpType


def build_program():
    nc = bacc.Bacc("TRN2", target_bir_lowering=False, debug=False, num_devices=N_CORES)

    xh = nc.dram_tensor("xh", [D, TOK], F8, kind="ExternalInput").ap()
    xl = nc.dram_tensor("xl", [D, TOK], F8, kind="ExternalInput").ap()
    # weights packed hi|lo along the output dim => 512B-contiguous DMA runs
    wq = nc.dram_tensor("wq", [D, 2 * CD], F8, kind="ExternalInput").ap()
    wk = nc.dram_tensor("wk", [D, 2 * CD], F8, kind="ExternalInput").ap()
    wv = nc.dram_tensor("wv", [D, 2 * CD], F8, kind="ExternalInput").ap()
    wo = nc.dram_tensor("wo", [CD, 2 * D], F8, kind="ExternalInput").ap()
    bq = nc.dram_tensor("bq", [CD], F32, kind="ExternalInput").ap()
    bk = nc.dram_tensor("bk", [CD], F32, kind="ExternalInput").ap()
    out = nc.dram_tensor("out", [B, D, S], BF16, kind="ExternalOutput").ap()

    with tile.TileContext(nc) as tc:
        _build_tile(nc, tc, xh, xl, wq, wk, wv, wo, bq, bk, out)

    nc.compile()
    return nc


def _build_tile(nc, tc, xh, xl, wq, wk, wv, wo, bq, bk, out):
    import contextlib

    ctx = contextlib.ExitStack()
    with ctx:
        const = ctx.enter_context(tc.tile_pool(name="const", bufs=1))
        xpool = ctx.enter_context(tc.tile_pool(name="x", bufs=3))
        qkv = ctx.enter_context(tc.tile_pool(name="qkv", bufs=2))
        mt_p = ctx.enter_context(tc.tile_pool(name="mt", bufs=4))
        est_p = ctx.enter_context(tc.tile_pool(name="est", bufs=8))
        small = ctx.enter_context(tc.tile_pool(name="small", bufs=4))
        outp = ctx.enter_context(tc.tile_pool(name="outp", bufs=6))
        # PSUM budget (8 banks): mm 2x2 + acc 2x1 + o 2x1 = 8
        ps_mm = ctx.enter_context(tc.tile_pool(name="ps_mm", bufs=2, space="PSUM"))
        ps_acc = ctx.enter_context(tc.tile_pool(name="ps_acc", bufs=2, space="PSUM"))
        ps_o = ctx.enter_context(tc.tile_pool(name="ps_o", bufs=2, space="PSUM"))

        # ---- resident constants ----
        # staged so the first projection matmuls can start ASAP:
        # wq (by ci-halves), first x tile (by ci-quarters), then the rest.
        xhr = xh.rearrange("(c p) t -> p c t", p=128)
        xlr = xl.rearrange("(c p) t -> p c t", p=128)
        # packed weight tiles: free dims [KC, 2*CD] with hi at [:, c, 0:CD]
        # and lo at [:, c, CD:2CD]
        wq_sb = const.tile([128, KC, 2 * CD], F8, tag="wq")
        wk_sb = const.tile([128, KC, 2 * CD], F8, tag="wk")
        wv_sb = const.tile([128, KC, 2 * CD], F8, tag="wv")
        wo_sb = const.tile([128, HPC, 2 * D], F8, tag="wo")
        wqr = wq.rearrange("(c p) m -> p c m", p=128)
        wkr = wk.rearrange("(c p) m -> p c m", p=128)
        wvr = wv.rearrange("(c p) m -> p c m", p=128)

        # staged in first-use order: the q-projection of tile 0 streams
        # wq+x chunk pairs immediately; wk is first needed ~2.5us in, wv ~10us
        xt0h = xpool.tile([128, KC, TT], F8, tag="xth")
        xt0l = xpool.tile([128, KC, TT], F8, tag="xtl")
        # chunk-major supply order matching the interleaved q/k consumption:
        # per group deliver x(hi,lo) then wq then wk, so the PE streams
        # chunk-pairs as they land. wv follows (first needed ~10us in).
        for lo, hi in ((0, 2), (2, 6), (6, 11), (11, 16)):
            nc.sync.dma_start(xt0h[:, lo:hi, :], xhr[:, lo:hi, 0:TT])
            nc.sync.dma_start(xt0l[:, lo:hi, :], xlr[:, lo:hi, 0:TT])
            nc.sync.dma_start(wq_sb[:, lo:hi, :], wqr[:, lo:hi, :])
            nc.sync.dma_start(wk_sb[:, lo:hi, :], wkr[:, lo:hi, :])
        for lo, hi in ((0, 8), (8, 16)):
            nc.sync.dma_start(wv_sb[:, lo:hi, :], wvr[:, lo:hi, :])

        bq_sb = const.tile([128, HPC], F32, tag="bq")
        nc.sync.dma_start(bq_sb[:], bq.rearrange("(h p) -> p h", p=128))
        bk_sb = const.tile([128, HPC], F32, tag="bk")
        nc.sync.dma_start(bk_sb[:], bk.rearrange("(h p) -> p h", p=128))

        # ones scaled by 1/SM so dn = sum(est)/SM and recip = SM/sum(est);
        # the MT tensor_mul then directly yields SM-scaled merged values,
        # letting Pool produce the fp8 hi/lo split with plain copy+sub.
        ones_sb = const.tile([128, 128], BF16, tag="ones")
        nc.vector.memset(ones_sb[:], 1.0 / SM)

        pending_p3 = []  # [generator, birth_qt, steps_left]
        qtctr = [0]  # global q-tile counter

        def inject(kp, h, last_qt):
            """Emit out-projection steps from the oldest pending generator.
            Steady-state split drain: an age-1 generator gives ~7 steps in
            the q-tile's second half (its MT8 is ready by then); the
            leftovers run as age-2 during the NEXT q-tile's first half, so
            every half-tile gets PE filler. kp7 boost covers the
            est7->dacc->join->dn latency at each h's end."""
            if not pending_p3:
                return
            age = qtctr[0] - pending_p3[0][1]
            if last_qt:
                # back-load the remaining filler toward the end-of-kernel
                # serial chain (est7->dacc->dn->recip->MT8)
                n = (3 if kp == NB // 2 - 1 else 1) if (h == 1 or kp >= 4) else 0
            elif age >= 2:
                n = 3 if kp == NB // 2 - 1 else 1
            elif age == 1 and h == 1 and kp < NB // 2 - 1:
                n = 1
            else:
                n = 0
            for _ in range(n):
                if not pending_p3:
                    return
                ent = pending_p3[0]
                if qtctr[0] - ent[1] < 1 and not last_qt:
                    return
                if next(ent[0], None) is None:
                    pending_p3.pop(0)
                else:
                    ent[2] -= 1

        states = {}

        def get_state(b):
            if b not in states:
                states[b] = {
                    "QT": qkv.tile([128, HPC, S], BF16, tag="QT", name=f"QT{b}"),
                    "KT": qkv.tile([128, HPC, S], BF16, tag="KT", name=f"KT{b}"),
                    "V": qkv.tile([128, NB, CD], BF16, tag="V", name=f"V{b}"),
                }
            return states[b]

        def mm3(ps, w_pair, x_pair, c, **kw):
            """3-term compensated fp8 DoubleRow accumulation for 256-chunk c.

            w_pair/x_pair: (hi_slice_fn, lo_slice_fn) returning the [128,2,*]
            APs for chunk c. Emits hi*hi, lo*hi, hi*lo.
            """
            wh, wl = w_pair
            xh_, xl_ = x_pair
            first = kw.pop("first")
            last = kw.pop("last")
            nc.tensor.matmul(ps, wh(c), xh_(c), start=first, stop=False, perf_mode=DR)
            nc.tensor.matmul(ps, wl(c), xh_(c), start=False, stop=False, perf_mode=DR)
            nc.tensor.matmul(ps, wh(c), xl_(c), start=False, stop=last, perf_mode=DR)

        def p1_tile(b, t):
            st = get_state(b)
            QT, KT, V = st["QT"], st["KT"], st["V"]
            off = b * S + t * TT
            if b == 0 and t == 0:
                xth, xtl = xt0h, xt0l
            else:
                xth = xpool.tile([128, KC, TT], F8, tag="xth")
                xtl = xpool.tile([128, KC, TT], F8, tag="xtl")
                nc.sync.dma_start(xth[:], xhr[:, :, off : off + TT])
                nc.sync.dma_start(xtl[:], xlr[:, :, off : off + TT])

            def xp(c):
                return xth[:, 2 * c : 2 * c + 2, :]

            def xpl(c):
                return xtl[:, 2 * c : 2 * c + 2, :]

            for h in range(HPC):
                mo = h * HD
                # q and k interleaved per chunk-pair so PE consumption
                # matches DMA arrival order (kills tile-0 startup stalls);
                # each keeps its own PSUM buf
                q_ps = ps_mm.tile([128, TT], F32, tag="mm")
                k_ps = ps_mm.tile([128, TT], F32, tag="mm")

                def wsl(w_sb, c, lo_half):
                    off = CD + mo if lo_half else mo
                    return w_sb[:, 2 * c : 2 * c + 2, off : off + HD]

                for c in range(KC2):
                    for w_sb, p_ps in ((wq_sb, q_ps), (wk_sb, k_ps)):
                        mm3(
                            p_ps[:],
                            (
                                lambda cc, _w=w_sb: wsl(_w, cc, False),
                                lambda cc, _w=w_sb: wsl(_w, cc, True),
                            ),
                            (xp, xpl),
                            c,
                            first=(c == 0),
                            last=(c == KC2 - 1),
                        )
                for p_ps, bias_sb, dst in (
                    (q_ps, bq_sb, QT),
                    (k_ps, bk_sb, KT),
                ):
                    nc.scalar.activation(
                        dst[:, h, t * TT : (t + 1) * TT],
                        p_ps[:],
                        Act.Identity,
                        bias=bias_sb[:, h : h + 1],
                        scale=INV_XW,
                    )
            for tb in range(TT // 128):
                v_ps = ps_acc.tile([128, CD], F32, tag="acc")

                def xsp(c, _tb=tb):
                    return xth[:, 2 * c : 2 * c + 2, _tb * 128 : (_tb + 1) * 128]

                def xspl(c, _tb=tb):
                    return xtl[:, 2 * c : 2 * c + 2, _tb * 128 : (_tb + 1) * 128]

                def wvp(c):
                    return wv_sb[:, 2 * c : 2 * c + 2, 0:CD]

                def wvpl(c):
                    return wv_sb[:, 2 * c : 2 * c + 2, CD : 2 * CD]

                for c in range(KC2):
                    # stationary = x chunk, moving = wv (note operand swap)
                    mm3(
                        v_ps[:],
                        (xsp, xspl),
                        (wvp, wvpl),
                        c,
                        first=(c == 0),
                        last=(c == KC2 - 1),
                    )
                nc.vector.tensor_scalar_mul(
                    V[:, t * (TT // 128) + tb, :], v_ps[:], INV_XW
                )

        for b in range(B):
            for t in range(NT):
                p1_tile(b, t)
                if b == 0 and t == 1:
                    nc.sync.dma_start(
                        wo_sb[:], wo.rearrange("(h p) m -> p h m", p=128)
                    )
            QT, KT, V = (get_state(b)[k] for k in ("QT", "KT", "V"))
            states.pop(b - 1, None)
            # ---- attention per q tile, P3 of previous q tile injected ----
            for qt in range(NQ):
                qsl = slice(qt * TT, (qt + 1) * TT)
                last_qt_f = b == B - 1 and qt == NQ - 1
                MT = mt_p.tile([128, HPC, TT], BF16, tag="MT")  # merged^T slice
                MT8h = mt_p.tile([128, HPC, TT], F8, tag="MT8h")
                MT8l = mt_p.tile([128, HPC, TT], F8, tag="MT8l")
                for h in range(HPC):
                    attn_ps = ps_acc.tile([128, TT], F32, tag="acc")
                    # two parallel denominator partial sums: DVE owns kp
                    # {0,3..7}, Pool owns {1,2} (finishes early, no chain lag)
                    dacc = small.tile([128, 2, TT], BF16, tag="dacc")
                    daccP = (
                        None
                        if last_qt_f
                        else small.tile([128, 2, TT], BF16, tag="daccP")
                    )
                    ests = [None] * (NB // 2)
                    st_tiles = [None] * (NB // 2)

                    def av_pair(kp):
                        for j in range(2):
                            kb = 2 * kp + j
                            nc.tensor.matmul(
                                attn_ps[:],
                                V[:, kb, h * HD : (h + 1) * HD],
                                ests[kp][:, j, :],
                                start=(kb == 0),
                                stop=(kb == NB - 1),
                            )

                    for kp in range(NB // 2):
                        # inject p3 BEFORE this iteration's dacc emission so
                        # its DVE copies run during the est wait, not behind
                        # it (engine queues are in-order)
                        last_qt = b == B - 1 and qt == NQ - 1
                        inject(kp, h, last_qt)
                        st_ps = ps_mm.tile([128, 2, TT], F32, tag="mm")
                        st_tiles[kp] = st_ps
                        for j in range(2):
                            kb = 2 * kp + j
                            nc.tensor.matmul(
                                st_ps[:, j, :],
                                KT[:, h, kb * 128 : (kb + 1) * 128],
                                QT[:, h, qsl],
                                start=True,
                                stop=True,
                            )
                        est = est_p.tile([128, 2, TT], BF16, tag="est")
                        nc.scalar.activation(est[:], st_ps[:], Act.Exp, scale=SCALE)
                        ests[kp] = est
                        if kp == 0:
                            nc.vector.tensor_copy(dacc[:], est[:])
                        elif kp in (1, 2, 3) and not last_qt_f:
                            if kp == 1:
                                nc.gpsimd.tensor_copy(daccP[:], est[:])
                            else:
                                nc.gpsimd.tensor_add(daccP[:], daccP[:], est[:])
                        else:
                            nc.vector.tensor_add(dacc[:], dacc[:], est[:])
                        if kp > 1:
                            av_pair(kp - 2)
                    av_pair(NB // 2 - 2)
                    av_pair(NB // 2 - 1)
                    if not last_qt_f:
                        nc.vector.tensor_add(dacc[:], dacc[:], daccP[:])
                    dn_ps = ps_o.tile([128, TT], F32, tag="o")
                    nc.tensor.matmul(
                        dn_ps[:], ones_sb[:], dacc[:, 0, :], start=True, stop=False
                    )
                    nc.tensor.matmul(
                        dn_ps[:], ones_sb[:], dacc[:, 1, :], start=False, stop=True
                    )
                    recip = small.tile([128, TT], F32, tag="recip")
                    nc.vector.reciprocal(recip[:], dn_ps[:])
                    nc.vector.tensor_mul(MT[:, h, :], attn_ps[:], recip[:])
                    if b == B - 1 and qt >= NQ - 2:
                        # endgame: convert each head's MT slice immediately so
                        # only half the split trails the final attention
                        nc.scalar.copy(MT8h[:, h, :], MT[:, h, :])
                        nc.gpsimd.tensor_sub(
                            MT8l[:, h, :], MT[:, h, :], MT8h[:, h, :]
                        )

                # MT is already SM-scaled; Pool does the fp8 hi/lo split
                # (2-qt p3 lag keeps it off the critical path)
                if not (b == B - 1 and qt >= NQ - 2):
                    nc.gpsimd.tensor_copy(MT8h[:], MT[:])
                    nc.gpsimd.tensor_sub(MT8l[:], MT[:], MT8h[:])

                endgame = b == B - 1 and qt >= NQ - 2
                # the very last generator runs post-attention: rotate its
                # PSUM tiles across all three pools (free by then) for a
                # 6-deep pipeline instead of ps_o's 2
                tail_pools = (
                    [(ps_o, "o"), (ps_mm, "mm"), (ps_acc, "acc")]
                    if b == B - 1 and qt == NQ - 1
                    else [(ps_o, "o")]
                )
                pending_p3.append(
                    [
                        _p3_steps(
                            nc, tail_pools, outp, wo_sb, MT8h, MT8l, out, b,
                            qsl, 2 if endgame else 6,
                        ),
                        qtctr[0],
                        D // 128,
                    ]
                )
                qtctr[0] += 1
        while pending_p3:
            for _ in pending_p3.pop(0)[0]:
                pass


def _p3_steps(nc, pools, outp, wo_sb, MT8h, MT8l, out, b, qsl, act_share=4):
    """Generator: one out-projection dblk per next() — injected between
    attention matmul pairs to fill PE gaps. 3-term fp8 DoubleRow over the
    CD=256 contraction (pair dim = the 2 heads)."""
    o_sb = None
    for dblk in range(D // 128):
        dsl = slice(dblk * 128, (dblk + 1) * 128)
        lsl = slice(D + dblk * 128, D + (dblk + 1) * 128)
        pool, tag = pools[dblk % len(pools)]
        o_ps = pool.tile([128, TT], F32, tag=tag, name=f"o_ps{b}_{dblk}")
        nc.tensor.matmul(
            o_ps[:], wo_sb[:, :, dsl], MT8h[:], start=True, stop=False, perf_mode=DR
        )
        nc.tensor.matmul(
            o_ps[:], wo_sb[:, :, lsl], MT8h[:], start=False, stop=False, perf_mode=DR
        )
        nc.tensor.matmul(
            o_ps[:], wo_sb[:, :, dsl], MT8l[:], start=False, stop=True, perf_mode=DR
        )
        if dblk % 2 == 0:
            o_sb = outp.tile([128, 2, TT], BF16, tag="o", name=f"o_sb{b}_{dblk}")
        if dblk % act_share == act_share - 1:
            nc.scalar.activation(o_sb[:, dblk % 2, :], o_ps[:], Act.Copy, scale=INV_MW)
        else:
            nc.vector.tensor_scalar_mul(o_sb[:, dblk % 2, :], o_ps[:], INV_MW)
        if dblk % 2 == 1:
            # one DMA per dblk pair: 2KB/partition clears the descriptor
            # floor and halves HWDGE descriptor load
            nc.sync.dma_start(
                out[b, (dblk - 1) * 128 : (dblk + 1) * 128, qsl].rearrange(
                    "(j p) s -> p j s", p=128
                ),
                o_sb[:],
            )
        yield dblk


_program = None


def _get_program():
    global _program
    if _program is None:
        _program = build_program()
    return _program


def _split8(a, scale):
    f8 = ml_dtypes.float8_e4m3
    a = np.asarray(a, np.float32) * scale
    hi = a.astype(f8)
    lo = (a - hi.astype(np.float32)).astype(f8)
    return hi, lo


def _split8_packed(a, scale):
    """hi|lo concatenated along the last axis."""
    hi, lo = _split8(a, scale)
    return np.concatenate([hi, lo], axis=-1)


def kernel(x, Wq, bq, Wk, bk, Wv, bv, Wo, bo):
    x = np.asarray(x, np.float32)
    Wq, Wk, Wv, Wo = (np.asarray(w, np.float32) for w in (Wq, Wk, Wv, Wo))
    bq, bk, bv, bo = (np.asarray(v, np.float32) for v in (bq, bk, bv, bo))

    xT = np.ascontiguousarray(x.reshape(TOK, D).T)
    xh_np, xl_np = _split8(xT, SX)

    nc = _get_program()
    in_maps = []
    for c in range(N_CORES):
        sl = slice(c * CD, (c + 1) * CD)
        in_maps.append(
            {
                "xh": xh_np,
                "xl": xl_np,
                "wq": _split8_packed(np.ascontiguousarray(Wq[sl, :].T), SW),
                "wk": _split8_packed(np.ascontiguousarray(Wk[sl, :].T), SW),
                "wv": _split8_packed(np.ascontiguousarray(Wv[sl, :].T), SW),
                "wo": _split8_packed(np.ascontiguousarray(Wo[:, sl].T), SW),
                "bq": np.ascontiguousarray(bq[sl]),
                "bk": np.ascontiguousarray(bk[sl]),
            }
        )

    res = run_bass_kernel_spmd(nc, in_maps, core_ids=list(range(N_CORES)))
    acc = np.zeros((B, D, S), np.float32)
    for r in res.results:
        acc += np.asarray(r["out"], np.float32)
    return np.ascontiguousarray(acc.transpose(0, 2, 1)) + (bo + Wo @ bv)


# revision 59
# speedup vs baseline: 1.0090x; 1.0090x over previous
"""Multi-head attention (B=2, S=2048, D=2048, H=16) on 8 trn2 NeuronCores.

Sharding: tensor-parallel over heads. Core c owns heads [2c, 2c+1]:
  - q/k/v projections via error-compensated fp8 DoubleRow matmuls
    (x and W split into e4m3 hi+lo pairs on host; 3 of 4 cross terms
    on-chip => bf16-level accuracy at 0.75x the PE cycles)
  - attention for its 2 heads x 2 batches in bf16 (transposed-score
    layout, fused softmax denominator, no on-chip transposes)
  - partial output projection (fp8 DoubleRow 3-term, MT split on-chip)
    merged_c @ Wo[:, c_slice].T  -> [B, D, S] bf16 partials
Host: sums the 8 partials, transposes, adds bo + Wo@bv (the v-bias
passes through attention unchanged since softmax weights sum to 1).

Engine split per attention q-tile: PE (scores/AV/dn/out-proj ~19.6us)
is the bottleneck; Act does the exp stream (~16.6us), DVE the softmax
denominator + PSUM readouts (~15us), Pool (gpsimd) the fp8 MT split +
part of the denominator accumulation. Out-projection "p3" steps are
injected into later q-tiles' PE gaps with age-based pacing.

Scaling: x is shipped as fp8(16*x) hi/lo, weights as fp8(64*W) hi/lo;
the 1/1024 is folded into the PSUM-readout activation scale.
"""

import numpy as np

try:
    import concourse.bass as bass  # noqa: F401
except ImportError:  # pragma: no cover - fresh grading dir
    import sys

    sys.path.insert(0, "/opt/trn_rl_repo")

import ml_dtypes

import concourse.bacc as bacc
import concourse.mybir as mybir
import concourse.tile as tile
from concourse.bass_utils import run_bass_kernel_spmd

B, S, D, H = 2, 2048, 2048, 16
HD = D // H  # 128
N_CORES = 8
HPC = H // N_CORES  # heads per core = 2
CD = HPC * HD  # per-core projection dims = 256
TOK = B * S  # 4096

BF16 = mybir.dt.bfloat16
F8 = mybir.dt.float8e4
F32 = mybir.dt.float32
DR = mybir.MatmulPerfMode.DoubleRow

TT = 512  # token tile (free dim of most matmuls)
KC = D // 128  # contraction chunks for projections = 16
KC2 = KC // 2  # DoubleRow (256-deep) chunks = 8
NB = S // 128  # key blocks per batch = 16
NQ = S // TT  # q tiles per batch = 4
NT = S // TT  # token tiles per batch = 4
SCALE = 1.0 / float(np.sqrt(HD))
SX = 16.0  # host pre-scale on x
SW = 64.0  # host pre-scale on weights
INV_XW = 1.0 / (SX * SW)
SM = 64.0  # on-chip pre-scale on merged (MT)
INV_MW = 1.0 / (SM * SW)

Act = mybir.ActivationFunctionType


def build_program():
    nc = bacc.Bacc("TRN2", target_bir_lowering=False, debug=False, num_devices=N_CORES)

    xh = nc.dram_tensor("xh", [D, TOK], F8, kind="ExternalInput").ap()
    xl = nc.dram_tensor("xl", [D, TOK], F8, kind="ExternalInput").ap()
    # weights packed hi|lo along the output dim => 512B-contiguous DMA runs
    wq = nc.dram_tensor("wq", [D, 2 * CD], F8, kind="ExternalInput").ap()
    wk = nc.dram_tensor("wk", [D, 2 * CD], F8, kind="ExternalInput").ap()
    wv = nc.dram_tensor("wv", [D, 2 * CD], F8, kind="ExternalInput").ap()
    wo = nc.dram_tensor("wo", [CD, 2 * D], F8, kind="ExternalInput").ap()
    bq = nc.dram_tensor("bq", [CD], F32, kind="ExternalInput").ap()
    bk = nc.dram_tensor("bk", [CD], F32, kind="ExternalInput").ap()
    out = nc.dram_tensor("out", [B, D, S], BF16, kind="ExternalOutput").ap()

    with tile.TileContext(nc) as tc:
        _build_tile(nc, tc, xh, xl, wq, wk, wv, wo, bq, bk, out)

    nc.compile()
    return nc


def _build_tile(nc, tc, xh, xl, wq, wk, wv, wo, bq, bk, out):
    import contextlib

    ctx = contextlib.ExitStack()
    with ctx:
        const = ctx.enter_context(tc.tile_pool(name="const", bufs=1))
        xpool = ctx.enter_context(tc.tile_pool(name="x", bufs=3))
        qkv = ctx.enter_context(tc.tile_pool(name="qkv", bufs=2))
        mt_p = ctx.enter_context(tc.tile_pool(name="mt", bufs=4))
        est_p = ctx.enter_context(tc.tile_pool(name="est", bufs=8))
        small = ctx.enter_context(tc.tile_pool(name="small", bufs=4))
        outp = ctx.enter_context(tc.tile_pool(name="outp", bufs=6))
        # PSUM budget (8 banks): mm 2x2 + acc 2x1 + o 2x1 = 8
        ps_mm = ctx.enter_context(tc.tile_pool(name="ps_mm", bufs=2, space="PSUM"))
        ps_acc = ctx.enter_context(tc.tile_pool(name="ps_acc", bufs=2, space="PSUM"))
        ps_o = ctx.enter_context(tc.tile_pool(name="ps_o", bufs=2, space="PSUM"))

        # ---- resident constants ----
        # staged so the first projection matmuls can start ASAP:
        # wq (by ci-halves), first x tile (by ci-quarters), then the rest.
        xhr = xh.rearrange("(c p) t -> p c t", p=128)
        xlr = xl.rearrange("(c p) t -> p c t", p=128)
        # packed weight tiles: free dims [KC, 2*CD] with hi at [:, c, 0:CD]
        # and lo at [:, c, CD:2CD]
        wq_sb = const.tile([128, KC, 2 * CD], F8, tag="wq")
        wk_sb = const.tile([128, KC, 2 * CD], F8, tag="wk")
        wv_sb = const.tile([128, KC, 2 * CD], F8, tag="wv")
        wo_sb = const.tile([128, HPC, 2 * D], F8, tag="wo")
        wqr = wq.rearrange("(c p) m -> p c m", p=128)
        wkr = wk.rearrange("(c p) m -> p c m", p=128)
        wvr = wv.rearrange("(c p) m -> p c m", p=128)

        # staged in first-use order: the q-projection of tile 0 streams
        # wq+x chunk pairs immediately; wk is first needed ~2.5us in, wv ~10us
        xt0h = xpool.tile([128, KC, TT], F8, tag="xth")
        xt0l = xpool.tile([128, KC, TT], F8, tag="xtl")
        # x-tile-0 + wq staged by first-use; wk/wv follow
        for lo, hi in ((0, 2), (2, 5), (5, 9), (9, 16)):
            nc.sync.dma_start(wq_sb[:, lo:hi, :], wqr[:, lo:hi, :])
            nc.sync.dma_start(xt0h[:, lo:hi, :], xhr[:, lo:hi, 0:TT])
            nc.sync.dma_start(xt0l[:, lo:hi, :], xlr[:, lo:hi, 0:TT])
        for lo, hi in ((0, 5), (5, 10), (10, 16)):
            nc.sync.dma_start(wk_sb[:, lo:hi, :], wkr[:, lo:hi, :])
        for lo, hi in ((0, 8), (8, 16)):
            nc.sync.dma_start(wv_sb[:, lo:hi, :], wvr[:, lo:hi, :])

        bq_sb = const.tile([128, HPC], F32, tag="bq")
        nc.sync.dma_start(bq_sb[:], bq.rearrange("(h p) -> p h", p=128))
        bk_sb = const.tile([128, HPC], F32, tag="bk")
        nc.sync.dma_start(bk_sb[:], bk.rearrange("(h p) -> p h", p=128))

        # ones scaled by 1/SM so dn = sum(est)/SM and recip = SM/sum(est);
        # the MT tensor_mul then directly yields SM-scaled merged values,
        # letting Pool produce the fp8 hi/lo split with plain copy+sub.
        ones_sb = const.tile([128, 128], BF16, tag="ones")
        nc.vector.memset(ones_sb[:], 1.0 / SM)

        pending_p3 = []  # [generator, birth_qt, steps_left]
        qtctr = [0]  # global q-tile counter

        def inject(kp, h, last_qt):
            """Emit out-projection steps from the oldest pending generator.
            Steady-state split drain: an age-1 generator gives ~7 steps in
            the q-tile's second half (its MT8 is ready by then); the
            leftovers run as age-2 during the NEXT q-tile's first half, so
            every half-tile gets PE filler. kp7 boost covers the
            est7->dacc->join->dn latency at each h's end."""
            if not pending_p3:
                return
            age = qtctr[0] - pending_p3[0][1]
            if last_qt:
                # back-load the remaining filler toward the end-of-kernel
                # serial chain (est7->dacc->dn->recip->MT8)
                n = (3 if kp == NB // 2 - 1 else 1) if (h == 1 or kp >= 4) else 0
            elif age >= 2:
                n = 3 if kp == NB // 2 - 1 else 1
            elif age == 1 and h == 1 and kp < NB // 2 - 1:
                n = 1
            else:
                n = 0
            for _ in range(n):
                if not pending_p3:
                    return
                ent = pending_p3[0]
                if qtctr[0] - ent[1] < 1 and not last_qt:
                    return
                if next(ent[0], None) is None:
                    pending_p3.pop(0)
                else:
                    ent[2] -= 1

        states = {}

        def get_state(b):
            if b not in states:
                states[b] = {
                    "QT": qkv.tile([128, HPC, S], BF16, tag="QT", name=f"QT{b}"),
                    "KT": qkv.tile([128, HPC, S], BF16, tag="KT", name=f"KT{b}"),
                    "V": qkv.tile([128, NB, CD], BF16, tag="V", name=f"V{b}"),
                }
            return states[b]

        def mm3(ps, w_pair, x_pair, c, **kw):
            """3-term compensated fp8 DoubleRow accumulation for 256-chunk c.

            w_pair/x_pair: (hi_slice_fn, lo_slice_fn) returning the [128,2,*]
            APs for chunk c. Emits hi*hi, lo*hi, hi*lo.
            """
            wh, wl = w_pair
            xh_, xl_ = x_pair
            first = kw.pop("first")
            last = kw.pop("last")
            nc.tensor.matmul(ps, wh(c), xh_(c), start=first, stop=False, perf_mode=DR)
            nc.tensor.matmul(ps, wl(c), xh_(c), start=False, stop=False, perf_mode=DR)
            nc.tensor.matmul(ps, wh(c), xl_(c), start=False, stop=last, perf_mode=DR)

        def p1_tile(b, t):
            st = get_state(b)
            QT, KT, V = st["QT"], st["KT"], st["V"]
            off = b * S + t * TT
            if b == 0 and t == 0:
                xth, xtl = xt0h, xt0l
            else:
                xth = xpool.tile([128, KC, TT], F8, tag="xth")
                xtl = xpool.tile([128, KC, TT], F8, tag="xtl")
                nc.sync.dma_start(xth[:], xhr[:, :, off : off + TT])
                nc.sync.dma_start(xtl[:], xlr[:, :, off : off + TT])

            def xp(c):
                return xth[:, 2 * c : 2 * c + 2, :]

            def xpl(c):
                return xtl[:, 2 * c : 2 * c + 2, :]

            for h in range(HPC):
                mo = h * HD
                for w_sb, bias_sb, dst in (
                    (wq_sb, bq_sb, QT),
                    (wk_sb, bk_sb, KT),
                ):
                    p_ps = ps_mm.tile([128, TT], F32, tag="mm")

                    def wp(c, _w=w_sb):
                        return _w[:, 2 * c : 2 * c + 2, mo : mo + HD]

                    def wpl(c, _w=w_sb):
                        return _w[:, 2 * c : 2 * c + 2, CD + mo : CD + mo + HD]

                    for c in range(KC2):
                        mm3(
                            p_ps[:],
                            (wp, wpl),
                            (xp, xpl),
                            c,
                            first=(c == 0),
                            last=(c == KC2 - 1),
                        )
                    nc.scalar.activation(
                        dst[:, h, t * TT : (t + 1) * TT],
                        p_ps[:],
                        Act.Identity,
                        bias=bias_sb[:, h : h + 1],
                        scale=INV_XW,
                    )
            for tb in range(TT // 128):
                v_ps = ps_acc.tile([128, CD], F32, tag="acc")

                def xsp(c, _tb=tb):
                    return xth[:, 2 * c : 2 * c + 2, _tb * 128 : (_tb + 1) * 128]

                def xspl(c, _tb=tb):
                    return xtl[:, 2 * c : 2 * c + 2, _tb * 128 : (_tb + 1) * 128]

                def wvp(c):
                    return wv_sb[:, 2 * c : 2 * c + 2, 0:CD]

                def wvpl(c):
                    return wv_sb[:, 2 * c : 2 * c + 2, CD : 2 * CD]

                for c in range(KC2):
                    # stationary = x chunk, moving = wv (note operand swap)
                    mm3(
                        v_ps[:],
                        (xsp, xspl),
                        (wvp, wvpl),
                        c,
                        first=(c == 0),
                        last=(c == KC2 - 1),
                    )
                nc.vector.tensor_scalar_mul(
                    V[:, t * (TT // 128) + tb, :], v_ps[:], INV_XW
                )

        for b in range(B):
            for t in range(NT):
                p1_tile(b, t)
                if b == 0 and t == 1:
                    nc.sync.dma_start(
                        wo_sb[:], wo.rearrange("(h p) m -> p h m", p=128)
                    )
            QT, KT, V = (get_state(b)[k] for k in ("QT", "KT", "V"))
            states.pop(b - 1, None)
            # ---- attention per q tile, P3 of previous q tile injected ----
            for qt in range(NQ):
                qsl = slice(qt * TT, (qt + 1) * TT)
                last_qt_f = b == B - 1 and qt == NQ - 1
                MT = mt_p.tile([128, HPC, TT], BF16, tag="MT")  # merged^T slice
                MT8h = mt_p.tile([128, HPC, TT], F8, tag="MT8h")
                MT8l = mt_p.tile([128, HPC, TT], F8, tag="MT8l")
                for h in range(HPC):
                    attn_ps = ps_acc.tile([128, TT], F32, tag="acc")
                    # two parallel denominator partial sums: DVE owns kp
                    # {0,3..7}, Pool owns {1,2} (finishes early, no chain lag)
                    dacc = small.tile([128, 2, TT], BF16, tag="dacc")
                    daccP = (
                        None
                        if last_qt_f
                        else small.tile([128, 2, TT], BF16, tag="daccP")
                    )
                    ests = [None] * (NB // 2)
                    st_tiles = [None] * (NB // 2)

                    def av_pair(kp):
                        for j in range(2):
                            kb = 2 * kp + j
                            nc.tensor.matmul(
                                attn_ps[:],
                                V[:, kb, h * HD : (h + 1) * HD],
                                ests[kp][:, j, :],
                                start=(kb == 0),
                                stop=(kb == NB - 1),
                            )

                    for kp in range(NB // 2):
                        # inject p3 BEFORE this iteration's dacc emission so
                        # its DVE copies run during the est wait, not behind
                        # it (engine queues are in-order)
                        last_qt = b == B - 1 and qt == NQ - 1
                        inject(kp, h, last_qt)
                        st_ps = ps_mm.tile([128, 2, TT], F32, tag="mm")
                        st_tiles[kp] = st_ps
                        for j in range(2):
                            kb = 2 * kp + j
                            nc.tensor.matmul(
                                st_ps[:, j, :],
                                KT[:, h, kb * 128 : (kb + 1) * 128],
                                QT[:, h, qsl],
                                start=True,
                                stop=True,
                            )
                        est = est_p.tile([128, 2, TT], BF16, tag="est")
                        nc.scalar.activation(est[:], st_ps[:], Act.Exp, scale=SCALE)
                        ests[kp] = est
                        if kp == 0:
                            nc.vector.tensor_copy(dacc[:], est[:])
                        elif kp in (1, 2, 3) and not last_qt_f:
                            if kp == 1:
                                nc.gpsimd.tensor_copy(daccP[:], est[:])
                            else:
                                nc.gpsimd.tensor_add(daccP[:], daccP[:], est[:])
                        else:
                            nc.vector.tensor_add(dacc[:], dacc[:], est[:])
                        if kp > 1:
                            av_pair(kp - 2)
                    av_pair(NB // 2 - 2)
                    inject(NB // 2 - 1, h, last_qt_f)
                    av_pair(NB // 2 - 1)
                    if not last_qt_f:
                        nc.vector.tensor_add(dacc[:], dacc[:], daccP[:])
                    dn_ps = ps_o.tile([128, TT], F32, tag="o")
                    nc.tensor.matmul(
                        dn_ps[:], ones_sb[:], dacc[:, 0, :], start=True, stop=False
                    )
                    nc.tensor.matmul(
                        dn_ps[:], ones_sb[:], dacc[:, 1, :], start=False, stop=True
                    )
                    recip = small.tile([128, TT], F32, tag="recip")
                    nc.vector.reciprocal(recip[:], dn_ps[:])
                    nc.vector.tensor_mul(MT[:, h, :], attn_ps[:], recip[:])
                    if b == B - 1 and qt >= NQ - 2:
                        # endgame: convert each head's MT slice immediately so
                        # only half the split trails the final attention
                        nc.scalar.copy(MT8h[:, h, :], MT[:, h, :])
                        nc.gpsimd.tensor_sub(
                            MT8l[:, h, :], MT[:, h, :], MT8h[:, h, :]
                        )

                # MT is already SM-scaled; Pool does the fp8 hi/lo split
                # (2-qt p3 lag keeps it off the critical path)
                if not (b == B - 1 and qt >= NQ - 2):
                    nc.gpsimd.tensor_copy(MT8h[:], MT[:])
                    nc.gpsimd.tensor_sub(MT8l[:], MT[:], MT8h[:])

                endgame = b == B - 1 and qt >= NQ - 2
                # the very last generator runs post-attention: rotate its
                # PSUM tiles across all three pools (free by then) for a
                # 6-deep pipeline instead of ps_o's 2
                tail_pools = (
                    [(ps_o, "o"), (ps_mm, "mm"), (ps_acc, "acc")]
                    if b == B - 1 and qt == NQ - 1
                    else [(ps_o, "o")]
                )
                pending_p3.append(
                    [
                        _p3_steps(
                            nc, tail_pools, outp, wo_sb, MT8h, MT8l, out, b,
                            qsl, 2 if endgame else 6,
                        ),
                        qtctr[0],
                        D // 128,
                    ]
                )
                qtctr[0] += 1
        while pending_p3:
            for _ in pending_p3.pop(0)[0]:
                pass


def _p3_steps(nc, pools, outp, wo_sb, MT8h, MT8l, out, b, qsl, act_share=4):
    """Generator: one out-projection dblk per next() — injected between
    attention matmul pairs to fill PE gaps. 3-term fp8 DoubleRow over the
    CD=256 contraction (pair dim = the 2 heads)."""
    o_sb = None
    for dblk in range(D // 128):
        dsl = slice(dblk * 128, (dblk + 1) * 128)
        lsl = slice(D + dblk * 128, D + (dblk + 1) * 128)
        pool, tag = pools[dblk % len(pools)]
        o_ps = pool.tile([128, TT], F32, tag=tag, name=f"o_ps{b}_{dblk}")
        nc.tensor.matmul(
            o_ps[:], wo_sb[:, :, dsl], MT8h[:], start=True, stop=False, perf_mode=DR
        )
        nc.tensor.matmul(
            o_ps[:], wo_sb[:, :, lsl], MT8h[:], start=False, stop=False, perf_mode=DR
        )
        nc.tensor.matmul(
            o_ps[:], wo_sb[:, :, dsl], MT8l[:], start=False, stop=True, perf_mode=DR
        )
        if dblk % 2 == 0:
            o_sb = outp.tile([128, 2, TT], BF16, tag="o", name=f"o_sb{b}_{dblk}")
        if dblk % act_share == act_share - 1:
            nc.scalar.activation(o_sb[:, dblk % 2, :], o_ps[:], Act.Copy, scale=INV_MW)
        else:
            nc.vector.tensor_scalar_mul(o_sb[:, dblk % 2, :], o_ps[:], INV_MW)
        if dblk % 2 == 1:
            # one DMA per dblk pair: 2KB/partition clears the descriptor
            # floor and halves HWDGE descriptor load
            nc.sync.dma_start(
                out[b, (dblk - 1) * 128 : (dblk + 1) * 128, qsl].rearrange(
                    "(j p) s -> p j s", p=128
                ),
                o_sb[:],
            )
        yield dblk


_program = None


def _get_program():
    global _program
    if _program is None:
        _program = build_program()
    return _program


def _split8(a, scale):
    f8 = ml_dtypes.float8_e4m3
    a = np.asarray(a, np.float32) * scale
    hi = a.astype(f8)
    lo = (a - hi.astype(np.float32)).astype(f8)
    return hi, lo


def _split8_packed(a, scale):
    """hi|lo concatenated along the last axis."""
    hi, lo = _split8(a, scale)
    return np.concatenate([hi, lo], axis=-1)


def kernel(x, Wq, bq, Wk, bk, Wv, bv, Wo, bo):
    x = np.asarray(x, np.float32)
    Wq, Wk, Wv, Wo = (np.asarray(w, np.float32) for w in (Wq, Wk, Wv, Wo))
    bq, bk, bv, bo = (np.asarray(v, np.float32) for v in (bq, bk, bv, bo))

    xT = np.ascontiguousarray(x.reshape(TOK, D).T)
    xh_np, xl_np = _split8(xT, SX)

    nc = _get_program()
    in_maps = []
    for c in range(N_CORES):
        sl = slice(c * CD, (c + 1) * CD)
        in_maps.append(
            {
                "xh": xh_np,
                "xl": xl_np,
                "wq": _split8_packed(np.ascontiguousarray(Wq[sl, :].T), SW),
                "wk": _split8_packed(np.ascontiguousarray(Wk[sl, :].T), SW),
                "wv": _split8_packed(np.ascontiguousarray(Wv[sl, :].T), SW),
                "wo": _split8_packed(np.ascontiguousarray(Wo[:, sl].T), SW),
                "bq": np.ascontiguousarray(bq[sl]),
                "bk": np.ascontiguousarray(bk[sl]),
            }
        )

    res = run_bass_kernel_spmd(nc, in_maps, core_ids=list(range(N_CORES)))
    acc = np.zeros((B, D, S), np.float32)
    for r in res.results:
        acc += np.asarray(r["out"], np.float32)
    return np.ascontiguousarray(acc.transpose(0, 2, 1)) + (bo + Wo @ bv)
